# revision 10
# baseline (speedup 1.0000x reference)
"""Bass/Tile TRN2 kernel for nn_STBlock (temporal conv + LN + GATv2 + LN).

Design (8 NeuronCores, data-parallel over the T=32 timesteps, TL=4 per core):
  - Row layout: nodes on SBUF partitions, features on the free axis.
  - Temporal conv = 3 accumulated PE matmuls per (t, node-chunk); LN1 via
    bn_stats/bn_aggr; the x residual is recovered on-device by PE-transposing
    the already-loaded xT slice (nothing extra shipped).
  - GATv2: per-node incoming-edge dense tables (slot-major, degree-sorted
    node chunks with per-chunk max degree), one combined 4-graph bf16
    dma_gather of gl[src] rows per chunk; all softmax segment ops are
    fixed-stride free-axis reduces; no scatter anywhere.
  - Edge features: compact eaF tables [16, cols] shipped bf16; ee = We.T@eaF
    computed on device (PE) and transposed into row layout.
"""
import hashlib
from contextlib import ExitStack

import numpy as np
import ml_dtypes

import jax
from jax.sharding import Mesh, PartitionSpec
from jax.experimental.shard_map import shard_map

import concourse.bass as bass
import concourse.mybir as mybir
import concourse.tile as tile
from concourse import bacc, library_config
from concourse import bass2jax

F32 = mybir.dt.float32
BF16 = mybir.dt.bfloat16
I16 = mybir.dt.int16
AX = mybir.AxisListType
OP = mybir.AluOpType
AF = mybir.ActivationFunctionType

N, T, H, E, ED, KH, D, KW = 1000, 32, 128, 16000, 16, 4, 32, 3
NP, CH = 1024, 8
TL, NCORES = 4, 8
NEG = 0.2
BF = ml_dtypes.bfloat16


# ---------------------------------------------------------------- host tables
def _build_tables(edge_index, edge_attr):
    ei = np.asarray(edge_index).astype(np.int64)
    ea = np.asarray(edge_attr, np.float32)
    src0, dst0 = ei[0], ei[1]
    cnt = np.zeros(N, np.float32)
    np.add.at(cnt, dst0, 1.0)
    ssum = np.zeros((N, ED), np.float32)
    np.add.at(ssum, dst0, ea)
    loop_attr = ssum / np.maximum(cnt, 1.0)[:, None]
    eaF = np.concatenate([ea, loop_attr], 0)                 # [E+N, ED]
    src = np.concatenate([src0, np.arange(N)])
    dst = np.concatenate([dst0, np.arange(N)])

    deg = np.zeros(N, np.int64)
    np.add.at(deg, dst, 1)
    order = np.argsort(deg, kind="stable")
    perm = np.full(NP, -1, np.int64)
    new_of_old = np.zeros(N, np.int64)
    perm[24:] = order
    new_of_old[order] = np.arange(24, NP)

    e_order = np.argsort(dst, kind="stable")
    starts = np.zeros(N + 1, np.int64)
    np.cumsum(np.bincount(dst[e_order], minlength=N), out=starts[1:])

    Dcs, mask_cat, ea_cat, idx_cat = [], [], [], []
    for c in range(CH):
        ids = perm[c * 128:(c + 1) * 128]
        Dc = int(max(1, max((deg[o] for o in ids if o >= 0), default=1)))
        stab = np.zeros((128, Dc), np.int16)
        mtab = np.zeros((128, Dc), np.float32)
        atab = np.zeros((128, Dc, ED), np.float32)
        for nl in range(128):
            old = ids[nl]
            if old < 0:
                mtab[nl, 0] = 1.0
                continue
            eids = e_order[starts[old]:starts[old + 1]]
            stab[nl, :len(eids)] = new_of_old[src[eids]]
            mtab[nl, :len(eids)] = 1.0
            atab[nl, :len(eids)] = eaF[eids]
        Dcs.append(Dc)
        mask_cat.append(mtab)
        # slot-major [16, Dc*128]: col j*128+nl -> eaF features of that edge
        ea_cat.append(np.ascontiguousarray(atab.transpose(2, 1, 0)
                                           ).reshape(ED, Dc * 128))
        flat = stab.T.reshape(-1)                            # slot-major
        idx_cat.append(flat.reshape(-1, 16).T.astype(np.int16))   # [16, Dc*8]
    return dict(
        perm=perm, Dcs=Dcs,
        mask=np.concatenate(mask_cat, 1).astype(BF),         # [128, SD]
        eaT=np.concatenate(ea_cat, 1).astype(BF),            # [16, SD*128]
        idx=np.concatenate(idx_cat, 1),                      # [16, SD*8]
    )


def _host_x_prep(x, perm):
    xnew = np.zeros((NP, T, H), np.float32)
    real = perm >= 0
    xnew[real] = np.asarray(x, np.float32)[0][perm[real]]
    xTs = []
    for s in range(NCORES):
        t0 = s * TL
        xh = np.zeros((6, NP, H), np.float32)
        lo, hi = max(0, t0 - 1), min(T, t0 + TL + 1)
        xh[lo - (t0 - 1):hi - (t0 - 1)] = xnew[:, lo:hi, :].transpose(1, 0, 2)
        xTs.append(np.ascontiguousarray(xh.transpose(2, 0, 1)).astype(BF))
    return xTs                                                # [128, 6, NP] bf16


# ---------------------------------------------------------------- program
def _pk_layout(SD):
    """One packed bf16-typed blob: bf16 section, f32 section (x2 elems),
    int16 idx section. Offsets in bf16/int16 elements (f32 offs are even)."""
    lay, off = {}, 0
    for name, n in [("xT", 128 * 6 * NP), ("wk", 128 * KW * 128),
                    ("wet", ED * 128), ("identb", 128 * 128),
                    ("attb", 128 * 128), ("eaT", ED * SD * 128),
                    ("mask", 128 * SD)]:
        lay[name] = off
        off += n
    off += off % 2
    for name in ["Wl", "Wr", "ident", "cb", "l1g", "l1b", "l2g", "l2b", "gbr"]:
        lay[name] = off                     # bf16-unit offset of f32 data
        off += 2 * 128 * 128
    lay["idx"] = off
    off += 16 * SD * 8
    return lay, off


def _build_program(Dcs):
    SD = sum(Dcs)
    nc = bacc.Bacc("TRN2", target_bir_lowering=False, debug=False,
                   enable_asserts=False, num_devices=NCORES,
                   enable_partition_id=False)

    L16, W = _pk_layout(SD)
    d_pk = nc.dram_tensor("pk", [W], BF16, kind="ExternalInput")
    d_gld = nc.dram_tensor("gld", [NP, TL * 128], BF16, kind="Internal")
    d_out = nc.dram_tensor("out", [TL, CH, 128, 128], BF16, kind="ExternalOutput")

    t16 = d_pk.ap().tensor

    def V16(name, dims):
        return bass.AP(t16, L16[name], [list(d) for d in dims])

    def V32(name, dims):
        # dims given in f32 units; scale to bf16 units then bitcast back
        bd = [[s * 2, c] for s, c in dims[:-1]] + [[1, dims[-1][1] * 2]]
        return bass.AP(t16, L16[name], bd).bitcast(F32)

    def Vidx(dims, extra=0):
        return bass.AP(t16, L16["idx"] + extra,
                       [list(d) for d in dims]).bitcast(I16)

    with tile.TileContext(nc) as tc, ExitStack() as ctx:
        const = ctx.enter_context(tc.tile_pool(name="const", bufs=1))
        psum = ctx.enter_context(
            tc.tile_pool(name="psum", bufs=1, space=bass.MemorySpace.PSUM))
        lda = ctx.enter_context(tc.tile_pool(name="lda", bufs=3))
        sa = ctx.enter_context(tc.tile_pool(name="sa", bufs=3))
        sml = ctx.enter_context(tc.tile_pool(name="sml", bufs=6))
        big = ctx.enter_context(tc.tile_pool(name="big", bufs=2))
        w1p = ctx.enter_context(tc.tile_pool(name="w1p", bufs=1))
        eep = ctx.enter_context(tc.tile_pool(name="eep", bufs=2))
        gcp = ctx.enter_context(tc.tile_pool(name="gcp", bufs=1))
        mid = ctx.enter_context(tc.tile_pool(name="mid", bufs=3))
        outp = ctx.enter_context(tc.tile_pool(name="outp", bufs=4))

        nc.gpsimd.load_library(library_config.mlp)
        wl = const.tile([128, 128], F32)
        wr = const.tile([128, 128], F32)
        nc.sync.dma_start(wl[:], V32("Wl", [[128, 128], [1, 128]]))
        nc.sync.dma_start(wr[:], V32("Wr", [[128, 128], [1, 128]]))
        wk = const.tile([128, KW, 128], BF16)
        nc.sync.dma_start(wk[:], V16("wk", [[KW * 128, 128], [128, KW], [1, 128]]))
        wet = const.tile([ED, 128], BF16)
        nc.sync.dma_start(wet[:], V16("wet", [[128, ED], [1, 128]]))
        ident = const.tile([128, 128], F32)
        nc.sync.dma_start(ident[:], V32("ident", [[128, 128], [1, 128]]))
        identb = const.tile([128, 128], BF16)
        nc.sync.dma_start(identb[:], V16("identb", [[128, 128], [1, 128]]))

        def rep_vec(name):
            t = const.tile([128, 128], F32, tag=f"vec_{name}")
            nc.sync.dma_start(t[:], V32(name, [[128, 128], [1, 128]]))
            return t
        cb = rep_vec("cb")
        l1g = rep_vec("l1g")
        l1b = rep_vec("l1b")
        l2g = rep_vec("l2g")
        l2b = rep_vec("l2b")
        gbr = rep_vec("gbr")
        attb = const.tile([128, 128], BF16)
        nc.sync.dma_start(attb[:], V16("attb", [[128, 128], [1, 128]]))
        msk = const.tile([128, SD], BF16)
        nc.sync.dma_start(msk[:], V16("mask", [[SD, 128], [1, SD]]))
        epst = const.tile([128, 1], F32)
        nc.vector.memset(epst[:], 1e-5)
        zbias = const.tile([128, 1], F32)
        nc.vector.memset(zbias[:], 0.0)
        idxt = const.tile([128, SD * 8], I16)
        for r in range(8):
            nc.sync.dma_start(idxt[16 * r:16 * (r + 1), :],
                              Vidx([[SD * 8, 16], [1, SD * 8]]))
        x1T = const.tile([128, TL, NP], F32)

        def APm(tile_ap, extra_offset, dims):
            full = [list(tile_ap.ap[0])] + [list(d) for d in dims]
            return bass.AP(tile_ap.tensor, tile_ap.offset + extra_offset, full)

        def layer_norm(v, g_t, b_t, out_t):
            stats = sml.tile([128, 6], F32, tag="stats")
            mv = sml.tile([128, 2], F32, tag="mv")
            nc.vector.bn_stats(out=stats[:], in_=v[:])
            nc.vector.bn_aggr(out=mv[:], in_=stats[:])
            rstd = sml.tile([128, 1], F32, tag="rstd")
            nc.scalar.activation(rstd[:], mv[:, 1:2], AF.Sqrt, bias=epst[:])
            nc.vector.reciprocal(rstd[:], rstd[:])
            nc.vector.tensor_scalar(
                out=out_t[:], in0=v[:], scalar1=mv[:, 0:1], scalar2=rstd[:],
                op0=OP.subtract, op1=OP.mult)
            nc.vector.tensor_tensor(out=out_t[:], in0=out_t[:], in1=g_t[:], op=OP.mult)
            nc.vector.tensor_tensor(out=out_t[:], in0=out_t[:], in1=b_t[:], op=OP.add)

        # ---------------- stage A: conv + residual + LN1 -> x1T ----------------
        for t in range(TL):
            for c in range(CH):
                cs = slice(c * 128, (c + 1) * 128)
                xs = lda.tile([128, 3, 128], BF16, tag="xs")
                xsrc = bass.AP(t16, L16["xT"] + t * NP + c * 128,
                               [[6 * NP, 128], [NP, 3], [1, 128]])
                nc.sync.dma_start(xs[:], xsrc)
                ps = psum.tile([128, 128], F32, tag="acc", bufs=3)
                for k in range(KW):
                    nc.tensor.matmul(ps[:], xs[:, k, :], wk[:, k, :],
                                     start=(k == 0), stop=(k == KW - 1))
                pxr = psum.tile([128, 128], BF16, tag="trp", bufs=3)
                nc.tensor.transpose(pxr[:], xs[:, 1, :], identb[:])
                v = sa.tile([128, 128], F32, tag="v")
                nc.scalar.activation(v[:], pxr[:], AF.Copy)
                nc.vector.tensor_tensor(out=v[:], in0=v[:], in1=ps[:], op=OP.add)
                nc.vector.tensor_tensor(out=v[:], in0=v[:], in1=cb[:], op=OP.add)
                x1r = sa.tile([128, 128], F32, tag="x1r")
                layer_norm(v, l1g, l1b, x1r)
                pst = psum.tile([128, 128], F32, tag="trp", bufs=3)
                nc.tensor.transpose(pst[:], x1r[:], ident[:])
                nc.scalar.activation(x1T[:, t, cs], pst[:], AF.Copy)

        # ---------------- stage B1: gl/gr, write gl rows to DRAM ----------------
        gl_rows = d_gld.ap()
        grcs = []
        for c in range(CH):
            cs = slice(c * 128, (c + 1) * 128)
            glc = gcp.tile([128, TL, 128], BF16, tag="glc", bufs=2)
            grc = gcp.tile([128, TL, 128], BF16, tag="grc", bufs=CH)
            grcs.append(grc)
            for g in range(TL):
                p1 = psum.tile([128, 128], F32, tag="acc", bufs=3)
                nc.tensor.matmul(p1[:], x1T[:, g, cs], wl[:], start=True, stop=True)
                nc.scalar.activation(glc[:, g, :], p1[:], AF.Copy)
                p2 = psum.tile([128, 128], F32, tag="acc", bufs=3)
                nc.tensor.matmul(p2[:], x1T[:, g, cs], wr[:], start=True, stop=True)
                nc.scalar.activation(grc[:, g, :], p2[:], AF.Copy)
            nc.sync.dma_start(d_gld.ap()[cs], glc[:].rearrange("p g h -> p (g h)"))

        # ---------------- stage B2: per-chunk edge stage + LN2 ----------------
        for c in range(CH):
            grc = grcs[c]
            Dc = Dcs[c]
            off = sum(Dcs[:c])
            # --- on-device ee: [128h, cols] = WeT.T @ eaT, transposed to rows
            eat = eep.tile([ED, Dc * 128], BF16, tag="eat")
            easrc = bass.AP(t16, L16["eaT"] + off * 128,
                            [[SD * 128, ED], [1, Dc * 128]])
            nc.sync.dma_start(eat[:], easrc)
            ee_t = eep.tile([128, Dc, 128], BF16, tag="ee")
            nb = (Dc * 128 + 511) // 512
            for b in range(nb):
                w = min(512, Dc * 128 - b * 512)
                pse = psum.tile([128, 512], F32, tag="acc", bufs=3)
                nc.tensor.matmul(pse[:, :w], wet[:], eat[:, b * 512:b * 512 + w],
                                 start=True, stop=True)
                ecol = sa.tile([128, 512], BF16, tag="ecol")
                nc.scalar.activation(ecol[:, :w], pse[:, :w], AF.Copy)
                for jj in range(w // 128):
                    j = b * 4 + jj
                    pet = psum.tile([128, 128], BF16, tag="trp", bufs=3)
                    nc.tensor.transpose(pet[:], ecol[:, jj * 128:(jj + 1) * 128],
                                        identb[:])
                    nc.scalar.activation(ee_t[:, j, :], pet[:], AF.Copy)
            # --- gather gl[src] for all 4 graphs
            gt = big.tile([128, Dc, 512], BF16, tag="gt")
            nc.gpsimd.dma_gather(
                gt[:], gl_rows, idxt[:, off * 8:(off + Dc) * 8],
                num_idxs=Dc * 128, num_idxs_reg=Dc * 128,
                elem_size=512, elem_step=512, single_packet=False)
            w1 = w1p.tile([128, Dc, 512], BF16, tag="w1")
            # s_pre = gt + gr (bcast j) + ee (bcast g)
            grb = APm(grc[:], 0, [[0, Dc], [1, 512]])
            nc.vector.tensor_tensor(out=w1[:], in0=gt[:], in1=grb, op=OP.add)
            w1_4 = w1[:].rearrange("p j (g h) -> p j g h", g=TL)
            eeb = APm(ee_t[:], 0, [[128, Dc], [0, TL], [1, 128]])
            nc.vector.tensor_tensor(out=w1_4, in0=w1_4, in1=eeb, op=OP.add)
            # leaky relu fused: max(x, 0.2x)
            w1f = w1[:].rearrange("p j e -> p (j e)")
            nc.vector.scalar_tensor_tensor(
                out=w1f, in0=w1f, scalar=NEG, in1=w1f, op0=OP.mult, op1=OP.max)
            # * att (bcast j,g), reduce d -> logits
            w1_5 = w1[:].rearrange("p j (g k d) -> p j g k d", g=TL, k=KH)
            attbb = APm(attb[:], 0, [[0, Dc], [0, TL], [32, KH], [1, D]])
            nc.vector.tensor_tensor(out=w1_5, in0=w1_5, in1=attbb, op=OP.mult)
            lg = mid.tile([128, Dc, TL, KH], F32, tag="lg")
            nc.vector.tensor_reduce(out=lg[:], in_=w1_5, axis=AX.X, op=OP.add)
            # w = exp(logits)*mask; den = sum_j; alpha = w/den
            we = mid.tile([128, Dc, TL * KH], F32, tag="we")
            nc.scalar.activation(we[:].rearrange("p j e -> p (j e)"),
                                 lg[:].rearrange("p j g k -> p (j g k)"),
                                 AF.Exp, bias=zbias[:])
            mskb = APm(msk[:], off, [[1, Dc], [0, TL * KH]])
            nc.vector.tensor_tensor(out=we[:], in0=we[:], in1=mskb, op=OP.mult)
            den = mid.tile([128, TL * KH], F32, tag="den")
            we_T = APm(we[:], 0, [[1, TL * KH], [TL * KH, Dc]])
            nc.vector.tensor_reduce(out=den[:], in_=we_T, axis=AX.X, op=OP.add)
            nc.vector.reciprocal(den[:], den[:])
            rdb = APm(den[:], 0, [[0, Dc], [1, TL * KH]])
            nc.vector.tensor_tensor(out=we[:], in0=we[:], in1=rdb, op=OP.mult)
            wab = mid.tile([128, Dc, TL * KH], BF16, tag="wab")
            nc.scalar.activation(wab[:].rearrange("p j e -> p (j e)"),
                                 we[:].rearrange("p j e -> p (j e)"), AF.Copy)
            # values *= alpha (bcast d); aggr = sum_j
            gt_5 = gt[:].rearrange("p j (g k d) -> p j g k d", g=TL, k=KH)
            wabb = APm(wab[:], 0, [[TL * KH, Dc], [KH, TL], [1, KH], [0, D]])
            nc.vector.tensor_tensor(out=gt_5, in0=gt_5, in1=wabb, op=OP.mult)
            ag = mid.tile([128, TL, 128], F32, tag="ag")
            ag_v = ag[:].rearrange("p g (k d) -> p g k d", k=KH)
            gt_T = APm(gt[:], 0, [[128, TL], [32, KH], [1, D], [512, Dc]])
            nc.vector.tensor_reduce(out=ag_v, in_=gt_T, axis=AX.X, op=OP.add)

            for g in range(TL):
                pst = psum.tile([128, 128], F32, tag="trp", bufs=3)
                x1slice = APm(x1T[:], g * NP + c * 128, [[1, 128]])
                nc.tensor.transpose(pst[:], x1slice, ident[:])
                v2 = outp.tile([128, 128], F32, tag="v2")
                nc.vector.tensor_tensor(out=v2[:], in0=ag[:, g, :], in1=pst[:],
                                        op=OP.add)
                nc.vector.tensor_tensor(out=v2[:], in0=v2[:], in1=gbr[:], op=OP.add)
                o = outp.tile([128, 128], BF16, tag="o")
                layer_norm(v2, l2g, l2b, o)
                nc.sync.dma_start(d_out.ap()[g, c], o[:])

    nc.compile()
    return nc


# ---------------------------------------------------------------- exec (pjrt)
class _Exec:
    """Persistent jitted shard_map executor around the compiled Bass module."""

    def __init__(self, nc):
        bass2jax.install_neuronx_cc_hook()
        self.nc = nc
        in_names, out_names, out_avals, zero_outs = [], [], [], []
        for alloc in nc.m.functions[0].allocations:
            if not isinstance(alloc, mybir.MemoryLocationSet):
                continue
            name = alloc.memorylocations[0].name
            if alloc.kind == "ExternalInput":
                in_names.append(name)
            elif alloc.kind == "ExternalOutput":
                out_names.append(name)
                shape = tuple(alloc.tensor_shape)
                dtype = mybir.dt.np(alloc.dtype)
                out_avals.append(jax.core.ShapedArray(shape, dtype))
                zero_outs.append(np.zeros(shape, dtype))
        self.in_names, self.out_names = in_names, out_names
        self.out_avals, self.zero_outs = out_avals, zero_outs
        n_params, n_outs = len(in_names), len(out_names)

        def _body(*args):
            outs = bass2jax._bass_exec_p.bind(
                *args,
                out_avals=tuple(out_avals),
                in_names=tuple(in_names),
                out_names=tuple(out_names),
                lowering_input_output_aliases=(),
                sim_require_finite=True,
                sim_require_nnan=True,
                nc=nc,
            )
            return tuple(outs)

        devices = jax.devices()[:NCORES]
        mesh = Mesh(np.asarray(devices), ("core",))
        self._fn = jax.jit(shard_map(
            _body, mesh=mesh,
            in_specs=(PartitionSpec("core"),) * n_params,
            out_specs=(PartitionSpec("core"),) * n_outs,
            check_rep=False))
        self._staged = None
        self._staged_key = None

    def stage(self, in_maps):
        pid_name = (self.nc.partition_id_tensor.name
                    if self.nc.partition_id_tensor else None)
        in_maps = [dict(m) for m in in_maps]
        for c in range(NCORES):
            if pid_name is not None:
                in_maps[c][pid_name] = np.array([[c]], dtype=np.uint32)
        key = hashlib.md5(
            b"".join(np.ascontiguousarray(in_maps[c][n]).tobytes()
                     for c in range(NCORES) for n in self.in_names)).hexdigest()
        if self._staged_key == key:
            return
        concat = [np.concatenate([np.asarray(in_maps[c][n]) for c in range(NCORES)],
                                 axis=0) for n in self.in_names]
        self._staged = [jax.device_put(a) for a in concat]
        jax.block_until_ready(self._staged)
        self._staged_key = key

    def run(self):
        outs = self._fn(*self._staged)
        jax.block_until_ready(outs)
        return outs

    def run_unpacked(self):
        outs = self.run()
        res = []
        for c in range(NCORES):
            res.append({
                name: np.asarray(outs[i]).reshape(
                    NCORES, *self.out_avals[i].shape)[c]
                for i, name in enumerate(self.out_names)})
        return res


_CACHE = {}


def _get_state(inputs):
    key = hashlib.md5(
        np.ascontiguousarray(np.asarray(inputs["edge_index"])).tobytes()
        + np.ascontiguousarray(np.asarray(inputs["edge_attr"], np.float32)).tobytes()
    ).hexdigest()
    if key not in _CACHE:
        _CACHE.clear()
        tabs = _build_tables(inputs["edge_index"], inputs["edge_attr"])
        nc = _build_program(tabs["Dcs"])
        _CACHE[key] = (tabs, _Exec(nc))
    return _CACHE[key]


def _make_in_maps(inputs, tabs):
    SD = sum(tabs["Dcs"])
    xTs = _host_x_prep(np.asarray(inputs["x"], np.float32), tabs["perm"])
    L16, W = _pk_layout(SD)

    def rep(v):
        return np.broadcast_to(np.asarray(v, np.float32)[None, :], (128, 128))

    pk = np.zeros(W, BF)

    def put16(name, arr):
        a = np.ascontiguousarray(arr).astype(BF).reshape(-1)
        pk[L16[name]:L16[name] + a.size] = a

    def put32(name, arr):
        a = np.ascontiguousarray(arr, dtype=np.float32).reshape(-1).view(BF)
        pk[L16[name]:L16[name] + a.size] = a

    wkk = np.stack([np.ascontiguousarray(
        np.asarray(inputs["conv_w"], np.float32)[:, :, k].T)
        for k in range(KW)]).transpose(1, 0, 2)          # [h, k, n]
    put16("wk", wkk)
    put16("wet", np.asarray(inputs["We"], np.float32))
    put16("identb", np.eye(128, dtype=np.float32))
    put16("attb", rep(np.asarray(inputs["att"], np.float32).reshape(128)))
    put16("eaT", tabs["eaT"])
    put16("mask", tabs["mask"])
    put32("Wl", inputs["Wl"])
    put32("Wr", inputs["Wr"])
    put32("ident", np.eye(128, dtype=np.float32))
    put32("cb", rep(inputs["conv_b"]))
    put32("l1g", rep(inputs["ln1_g"]))
    put32("l1b", rep(inputs["ln1_b"]))
    put32("l2g", rep(inputs["ln2_g"]))
    put32("l2b", rep(inputs["ln2_b"]))
    put32("gbr", rep(inputs["gat_b"]))
    i = np.ascontiguousarray(tabs["idx"]).view(BF).reshape(-1)
    pk[L16["idx"]:L16["idx"] + i.size] = i

    maps = []
    for s in range(NCORES):
        pks = pk.copy()
        x = xTs[s].reshape(-1)
        pks[L16["xT"]:L16["xT"] + x.size] = x
        maps.append(dict(pk=pks))
    return maps


def _unpack_out(outs, perm):
    full = np.zeros((1, N, T, H), np.float32)
    real = perm >= 0
    idx = perm[real]
    for s in range(NCORES):
        o = np.asarray(outs[s]).astype(np.float32).reshape(TL, NP, H)
        for t in range(TL):
            full[0, idx, s * TL + t, :] = o[t][real]
    return full


def kernel(**inputs) -> np.ndarray:
    tabs, ex = _get_state(inputs)
    in_maps = _make_in_maps(inputs, tabs)
    ex.stage(in_maps)
    res = ex.run_unpacked()
    return _unpack_out([res[c]["out"] for c in range(NCORES)], tabs["perm"])
